# revision 1
# baseline (speedup 1.0000x reference)
"""AttnConv (GNN message passing) Trainium2 kernel.

Math: out[i] = sum_{e: dst_e=i} a_e * h[src_e], a = scatter-softmax(scores, dst),
scores = alpha_q[dst] + alpha_k[src] + b.  Within one dst group, alpha_q[dst]+b
is constant, so it cancels in the softmax:
    a_e = w[src_e] / sum_{e': dst=i} w[src_e'],   w = exp(alpha_k - C)
Hence out = (A @ (w*h)) / (A @ w) with A the edge incidence (dst x src, with
multiplicity).  Device work = gather G=[w*h] rows per edge (dma_gather) +
segment-sum over dst via one-hot matmuls accumulating in PSUM (output kept
transposed: psum[j, node] so the gathered chunk is the cheap stationary
operand).

Sharding: edges partitioned by dst range (12500 nodes per core), G table
replicated, no collectives.  Host does the (untimed) preprocessing: tiny
matvec for alpha_k, sort edges by (block, src-quarter, src), pad to 128-edge
chunks, and the final divide + transpose.
"""

import os

import numpy as np

import concourse.bacc as bacc
import concourse.bass as bass
import concourse.tile as tile
from concourse import mybir
from concourse.bass_utils import run_bass_kernel_spmd

N_NODES = 100000
D = 64
N_CORES = 8
P = 128
Q_ROWS = 102400  # no quartering: int32 indirect offsets

DTYPE = os.environ.get("GNN_DTYPE", "f32")  # f32 | f16
G_BLK = int(os.environ.get("GNN_GBLK", "4"))  # node-blocks per gather group

_FDT = {"f32": mybir.dt.float32, "f16": mybir.dt.float16}

last_results = None  # BassKernelResults of the most recent run (test harness)


def _preprocess(h, W_attn, edge_index, n_cores, n_nodes, d):
    """Host-side sharding/layout."""
    nc_nodes = n_nodes // n_cores
    nblk = (nc_nodes + P - 1) // P
    ngrp = -(-nblk // G_BLK)
    nq = -(-n_nodes // Q_ROWS)

    h = np.asarray(h, dtype=np.float32)
    W_attn = np.asarray(W_attn, dtype=np.float32)
    src = np.asarray(edge_index[0], dtype=np.int64)
    dst = np.asarray(edge_index[1], dtype=np.int64)

    alpha_k = h @ W_attn[d:, 0]
    w = np.exp(alpha_k - alpha_k.max()).astype(np.float32)
    if DTYPE == "f32":
        ew_g = d
        gtab = np.zeros((nq * Q_ROWS, ew_g), dtype=np.float32)
        gtab[:n_nodes] = h * w[:, None]
    else:
        ew_g = 2 * d
        gtab = np.zeros((nq * Q_ROWS, ew_g), dtype=np.float16)
        gtab[:n_nodes, :d] = (h * w[:, None]).astype(np.float16)
        gtab[:n_nodes, d] = w.astype(np.float16)

    core = dst // nc_nodes
    localdst = dst - core * nc_nodes
    blk = localdst >> 7
    grp = blk // G_BLK
    q = src // Q_ROWS
    order = np.lexsort((src, blk, q, grp, core))
    core_s = core[order]
    grp_s = grp[order]
    q_s = q[order]
    blk_s = blk[order]
    src_s = src[order].astype(np.int64)
    off_s = (localdst[order] & 127).astype(np.float32)

    # counts per (core, grp, q)
    cgq = (core_s * ngrp + grp_s) * nq + q_s
    counts = np.bincount(cgq, minlength=n_cores * ngrp * nq).reshape(
        n_cores, ngrp, nq
    )
    Kgq = -(-counts.max(axis=0) // P)  # [ngrp, nq] chunks per run (can be 0)
    Kg = Kgq.sum(axis=1)  # [ngrp]
    Kg_max = int(Kg.max())
    starts = np.zeros(n_cores * ngrp * nq + 1, dtype=np.int64)
    np.cumsum(counts.reshape(-1), out=starts[1:])

    # chunk column index of each run: runs ordered by q within a group
    runbase = np.zeros((ngrp, nq), dtype=np.int64)
    for g in range(ngrp):
        cb = 0
        for qq in range(nq):
            runbase[g, qq] = cb
            cb += Kgq[g, qq]

    # task columns per group: union over cores of (chunk col j, block b)
    # pairs, plus a dummy all(-1) column for blocks with no tasks.
    tasks = []  # tasks[g] = list of (j, b_local) in emission order
    blk_tasks = []  # blk_tasks[g][b_local] = list of task indices m
    for g in range(ngrp):
        nb = min(G_BLK, nblk - g * G_BLK)
        pairs = set()
        for c in range(n_cores):
            for qq in range(nq):
                base = (c * ngrp + g) * nq + qq
                s0, s1 = starts[base], starts[base + 1]
                if s1 == s0:
                    continue
                eblk = blk_s[s0:s1] - g * G_BLK
                echk = runbase[g, qq] + np.arange(s1 - s0) // P
                pairs.update(zip(echk.tolist(), eblk.tolist()))
        tl = sorted(pairs, key=lambda t: (t[1], t[0]))
        bt = [[] for _ in range(nb)]
        for m, (j, b) in enumerate(tl):
            bt[b].append(m)
        for b in range(nb):
            if not bt[b]:
                tl.append((0, b))
                bt[b].append(len(tl) - 1)
        tasks.append(tl)
        blk_tasks.append(bt)
    M_max = max(len(t) for t in tasks)

    # per-core aux arrays
    fnp = np.float32 if DTYPE == "f32" else np.float16
    aux_dst_pc = []
    aux_idx_pc = []
    aux_w_pc = []
    for c in range(n_cores):
        adst = np.full((ngrp, P, M_max), -1.0, dtype=fnp)
        aidx = np.zeros((ngrp, P, max(Kg_max, 1)), dtype=np.int32)
        aw = np.zeros((ngrp, P, max(Kg_max, 1)), dtype=np.float32)
        for g in range(ngrp):
            for qq in range(nq):
                kq = int(Kgq[g, qq])
                if kq == 0:
                    continue
                base = (c * ngrp + g) * nq + qq
                s0, s1 = starts[base], starts[base + 1]
                n_pad = kq * P
                ip = np.zeros(n_pad, dtype=np.int32)
                ip[: s1 - s0] = (src_s[s0:s1] - qq * Q_ROWS).astype(np.int32)
                cb = int(runbase[g, qq])
                aidx[g, :, cb : cb + kq] = ip.reshape(kq, P).T
                wp = np.zeros(n_pad, dtype=np.float32)
                wp[: s1 - s0] = w[src_s[s0:s1]]
                aw[g, :, cb : cb + kq] = wp.reshape(kq, P).T
            # dst one-hot columns per task
            for m, (j, b) in enumerate(tasks[g]):
                # which quarter run does chunk j belong to?
                qq = int(np.searchsorted(runbase[g], j, side="right") - 1)
                while qq + 1 < nq and Kgq[g, qq] == 0:
                    qq += 1
                base = (c * ngrp + g) * nq + qq
                s0, s1 = starts[base], starts[base + 1]
                jl = j - int(runbase[g, qq])
                e0 = s0 + jl * P
                n_real = max(0, min(P, (s1 - e0)))
                if n_real <= 0:
                    continue
                sel = slice(e0, e0 + n_real)
                col = np.full(P, -1.0, dtype=np.float32)
                mask = blk_s[sel] == g * G_BLK + b
                col[:n_real][mask] = off_s[sel][mask]
                adst[g, :, m] = col.astype(fnp)
        aux_dst_pc.append(adst)
        aux_idx_pc.append(aidx)
        aux_w_pc.append(aw)

    iota = np.tile(np.arange(P, dtype=fnp), (P, 1))
    meta = dict(
        nc_nodes=nc_nodes,
        nblk=nblk,
        ngrp=ngrp,
        nq=nq,
        Kg_max=max(Kg_max, 1),
        M_max=M_max,
        ew_g=ew_g,
        Kgq=Kgq,
        runbase=runbase,
        tasks=tasks,
        blk_tasks=blk_tasks,
    )
    return gtab, iota, aux_dst_pc, aux_idx_pc, aux_w_pc, meta


def _build_program(n_nodes, d, meta, n_cores):
    fdt = _FDT[DTYPE]
    nblk = meta["nblk"]
    ngrp = meta["ngrp"]
    nq = meta["nq"]
    Kg_max = meta["Kg_max"]
    M_max = meta["M_max"]
    ew_g = meta["ew_g"]
    Kgq = meta["Kgq"]
    runbase = meta["runbase"]
    tasks = meta["tasks"]
    blk_tasks = meta["blk_tasks"]

    nc = bacc.Bacc(
        "TRN2",
        target_bir_lowering=False,
        debug=False,
        enable_asserts=False,
        num_devices=n_cores,
    )
    gt = nc.dram_tensor("gtab", [nq * Q_ROWS, ew_g], fdt, kind="ExternalInput")
    adst = nc.dram_tensor("adst", [ngrp, P, M_max], fdt, kind="ExternalInput")
    aidx = nc.dram_tensor(
        "aidx", [ngrp, P, Kg_max], mybir.dt.int32, kind="ExternalInput"
    )
    if DTYPE == "f32":
        aw = nc.dram_tensor(
            "aw", [ngrp, P, Kg_max], mybir.dt.float32, kind="ExternalInput"
        )
    iot = nc.dram_tensor("iota", [P, P], fdt, kind="ExternalInput")
    outt = nc.dram_tensor(
        "outt", [d + 1, nblk * P], mybir.dt.float32, kind="ExternalOutput"
    )

    with tile.TileContext(nc) as tc:
        with (
            tc.tile_pool(name="const", bufs=1) as cpool,
            tc.tile_pool(name="auxp", bufs=3) as apool,
            tc.tile_pool(name="gath", bufs=2) as gpool,
            tc.tile_pool(name="sw", bufs=2) as spool,
            tc.tile_pool(name="ob", bufs=4) as opool,
            tc.tile_pool(name="ps", bufs=4, space="PSUM") as pspool,
        ):
            it = cpool.tile([P, P], fdt)
            nc.sync.dma_start(out=it[:], in_=iot[:, :])

            for g in range(ngrp):
                Mg = len(tasks[g])
                Kg = int(Kgq[g].sum())
                dst_t = apool.tile([P, M_max], fdt, tag="adst")
                nc.sync.dma_start(out=dst_t[:], in_=adst[g])
                idx_t = apool.tile([P, Kg_max], mybir.dt.int32, tag="aidx")
                nc.sync.dma_start(out=idx_t[:], in_=aidx[g])
                if DTYPE == "f32":
                    w_t = apool.tile([P, Kg_max], mybir.dt.float32, tag="aw")
                    nc.sync.dma_start(out=w_t[:], in_=aw[g])
                gtile = gpool.tile([P, Kg_max * ew_g], fdt, tag="gt")
                for qq in range(nq):
                    kq = int(Kgq[g, qq])
                    if kq == 0:
                        continue
                    cb = int(runbase[g, qq])
                    if os.environ.get("GNN_NO_GATHER"):
                        nc.vector.memset(
                            gtile[:, cb * ew_g : (cb + kq) * ew_g], 1.0
                        )
                        continue
                    for jc in range(cb, cb + kq):
                        nc.gpsimd.indirect_dma_start(
                            out=gtile[:, jc * ew_g : (jc + 1) * ew_g],
                            out_offset=None,
                            in_=gt[:, :],
                            in_offset=bass.IndirectOffsetOnAxis(
                                ap=idx_t[:, jc : jc + 1], axis=0
                            ),
                        )
                # batched one-hot build: sb[:, m*128+p] = (dst_t[:,m] == p)
                sb = spool.tile([P, M_max * P], fdt, tag="sw")
                if os.environ.get("GNN_NO_TT"):
                    nc.vector.memset(sb[:, 0 : Mg * P], 0.0)
                elif True:
                    nc.any.tensor_tensor(
                    out=sb[:, 0 : Mg * P].rearrange("p (m q) -> p m q", q=P),
                    in0=it[:].unsqueeze(1).to_broadcast([P, Mg, P]),
                    in1=dst_t[:, 0:Mg].unsqueeze(2).to_broadcast([P, Mg, P]),
                    op=mybir.AluOpType.is_equal,
                )
                nb = min(G_BLK, nblk - g * G_BLK)
                for b in range(nb):
                    tl = blk_tasks[g][b]
                    pst = pspool.tile([P, P], mybir.dt.float32, tag="ps")
                    if DTYPE == "f32":
                        psd = pspool.tile([P, P], mybir.dt.float32, tag="psd")
                    for i, m in enumerate(tl):
                        j = tasks[g][m][0]
                        first, last = i == 0, i == len(tl) - 1
                        rhs = sb[:, m * P : (m + 1) * P]
                        if DTYPE == "f32":
                            nc.tensor.matmul(
                                out=pst[0:d, :],
                                lhsT=gtile[:, j * ew_g : j * ew_g + d],
                                rhs=rhs,
                                start=first,
                                stop=last,
                            )
                            nc.tensor.matmul(
                                out=psd[0:1, :],
                                lhsT=w_t[:, j : j + 1],
                                rhs=rhs,
                                start=first,
                                stop=last,
                            )
                        else:
                            nc.tensor.matmul(
                                out=pst[0 : d + 1, :],
                                lhsT=gtile[:, j * ew_g : j * ew_g + d + 1],
                                rhs=rhs,
                                start=first,
                                stop=last,
                            )
                    ob = opool.tile([P, P], mybir.dt.float32, tag="ob")
                    if DTYPE == "f32":
                        nc.scalar.copy(out=ob[0:d, :], in_=pst[0:d, :])
                        nc.scalar.copy(out=ob[d : d + 1, :], in_=psd[0:1, :])
                    else:
                        nc.scalar.copy(out=ob[0 : d + 1, :], in_=pst[0 : d + 1, :])
                    bb = g * G_BLK + b
                    nc.sync.dma_start(
                        out=outt[:, bb * P : (bb + 1) * P], in_=ob[0 : d + 1, :]
                    )
    nc.compile()
    return nc


def _run(h, h_attn_q, W_attn, b_attn, edge_index, n_cores, n_nodes, d, **spmd_kwargs):
    global last_results
    gtab, iota, adst_pc, aidx_pc, aw_pc, meta = _preprocess(
        h, W_attn, edge_index, n_cores, n_nodes, d
    )
    nc = _build_program(n_nodes, d, meta, n_cores)
    in_maps = []
    for c in range(n_cores):
        m = {"gtab": gtab, "iota": iota, "adst": adst_pc[c], "aidx": aidx_pc[c]}
        if DTYPE == "f32":
            m["aw"] = aw_pc[c]
        in_maps.append(m)
    res = run_bass_kernel_spmd(
        nc, in_maps, core_ids=list(range(n_cores)), **spmd_kwargs
    )
    last_results = res
    if os.environ.get("GNN_TIME2"):
        import time as _time

        global last_exec_s
        t0 = _time.time()
        res = run_bass_kernel_spmd(
            nc, in_maps, core_ids=list(range(n_cores)), **spmd_kwargs
        )
        last_exec_s = _time.time() - t0
        last_results = res
    nc_nodes = meta["nc_nodes"]
    out = np.empty((n_nodes, d), dtype=np.float32)
    for c in range(n_cores):
        o = np.asarray(res.results[c]["outt"], dtype=np.float32)
        num = o[:d, :nc_nodes]
        s = o[d, :nc_nodes]
        out[c * nc_nodes : (c + 1) * nc_nodes] = (num / (s + 1e-16)).T
    return out


def kernel(h, h_attn_q, W_attn, b_attn, edge_index):
    return _run(h, h_attn_q, W_attn, b_attn, edge_index, N_CORES, N_NODES, D)



# revision 2
# speedup vs baseline: 4.5088x; 4.5088x over previous
"""AttnConv (GNN message passing) Trainium2 kernel — src-sharded edge-parallel.

Math: out[i] = sum_{e: dst_e=i} a_e * h[src_e], a = scatter-softmax(scores, dst),
scores = alpha_q[dst] + alpha_k[src] + b.  Within one dst group alpha_q[dst]+b is
constant and cancels in the softmax, so with w = exp(alpha_k - C):
    out[i] = (sum_e w[src_e] * h[src_e]) / (sum_e w[src_e])

Sharding (the axon host<->device tunnel is ~20-50 MB/s, so bytes moved per run
dominate):  edges are assigned to the core that OWNS their src row.  Each core
holds only its 1/8 slice of the gather table g = [w*h, w] (f16), gathers rows
per edge chunk, scatter-adds into a full-size partial [100352, 66] via one-hot
matmuls (PSUM), then one f16 ReduceScatter(add) leaves each core with its dst
range [c*12544, (c+1)*12544).  Host divides num/den and assembles.

Per-edge metadata is compressed: local src index int16 + dst offset uint8,
decompressed on device.  Edges are sorted by (dst block, src); each 128-node
dst block gets ceil(max_core_count/128) chunks of 128 edge slots; pad slots
point at a zeroed table row so they contribute nothing.
"""

import numpy as np

import concourse.bacc as bacc
import concourse.bass as bass
import concourse.tile as tile
from concourse import mybir
from concourse.bass_utils import run_bass_kernel_spmd

N_NODES = 100000
D = 64
N_CORES = 8
P = 128
NC_SRC = N_NODES // N_CORES          # 12500 table rows owned per core
NR = 12544                           # table rows padded (98 * 128)
PADROW = NR - 1                      # zeroed row used by pad slots
NRC = 12544                          # dst nodes per core chunk (= NPAD / 8)
NPAD = NRC * N_CORES                 # 100352 padded dst space
NBLK = NPAD // P                     # 784 dst blocks
EW = 66                              # table row: 64 w*h cols + w + pad
GB = 8                               # dst blocks per work batch

F16 = mybir.dt.float16

last_results = None  # BassKernelResults of the most recent run (test harness)


def _preprocess(h, W_attn, edge_index):
    h = np.asarray(h, dtype=np.float32)
    W = np.asarray(W_attn, dtype=np.float32)
    src = np.asarray(edge_index[0]).astype(np.int64)
    dst = np.asarray(edge_index[1]).astype(np.int64)

    alpha = h @ W[D:, 0]
    w = np.exp(alpha - alpha.max(), dtype=np.float32)
    gtab = np.zeros((N_CORES, NR, EW), dtype=np.float16)
    gtab[:, :NC_SRC, :D] = (h * w[:, None]).astype(np.float16).reshape(
        N_CORES, NC_SRC, D
    )
    gtab[:, :NC_SRC, D] = w.astype(np.float16).reshape(N_CORES, NC_SRC)

    core = src // NC_SRC
    blk = dst >> 7
    key = core * NBLK + blk
    order = np.lexsort((src, key))
    key_s = key[order]
    srcl_s = (src[order] - core[order] * NC_SRC).astype(np.int16)
    off_s = (dst[order] & 127).astype(np.uint8)

    cnt = np.bincount(key_s, minlength=N_CORES * NBLK).reshape(N_CORES, NBLK)
    Kb = np.maximum(1, -(-cnt.max(axis=0) // P)).astype(np.int64)  # [NBLK]
    taskofs = np.zeros(NBLK + 1, dtype=np.int64)
    np.cumsum(Kb, out=taskofs[1:])
    M = int(taskofs[-1])

    cstart = np.zeros(N_CORES * NBLK, dtype=np.int64)
    np.cumsum(cnt.reshape(-1)[:-1], out=cstart[1:])
    rank = np.arange(key_s.shape[0], dtype=np.int64) - cstart[key_s]
    slot = (taskofs[key_s % NBLK] << 7) + rank
    core_s = key_s // NBLK

    aidx = np.full((N_CORES, M * P), PADROW, dtype=np.int16)
    adst = np.zeros((N_CORES, M * P), dtype=np.uint8)
    aidx[core_s, slot] = srcl_s
    adst[core_s, slot] = off_s
    aidx = np.ascontiguousarray(aidx.reshape(N_CORES, M, P).transpose(0, 2, 1))
    adst = np.ascontiguousarray(adst.reshape(N_CORES, M, P).transpose(0, 2, 1))
    return gtab, aidx, adst, Kb, taskofs, M


def _build_program(M, Kb, taskofs):
    nc = bacc.Bacc(
        "TRN2",
        target_bir_lowering=False,
        debug=False,
        enable_asserts=False,
        num_devices=N_CORES,
    )
    gt = nc.dram_tensor("gtab", [NR, EW], F16, kind="ExternalInput")
    ai = nc.dram_tensor("aidx", [P, M], mybir.dt.int16, kind="ExternalInput")
    ad = nc.dram_tensor("adst", [P, M], mybir.dt.uint8, kind="ExternalInput")
    outt = nc.dram_tensor("outt", [NRC, EW], F16, kind="ExternalOutput")

    batches = []  # (b0, nb, t0, tb)
    for b0 in range(0, NBLK, GB):
        nb = min(GB, NBLK - b0)
        t0 = int(taskofs[b0])
        tb = int(taskofs[b0 + nb] - t0)
        batches.append((b0, nb, t0, tb))
    TBM = max(tb for _, _, _, tb in batches)

    with tile.TileContext(nc) as tc:
        with (
            tc.tile_pool(name="const", bufs=1) as cpool,
            tc.tile_pool(name="gath", bufs=3) as gpool,
            tc.tile_pool(name="oneh", bufs=3) as spool,
            tc.tile_pool(name="ob", bufs=4) as opool,
            tc.tile_pool(name="ps", bufs=8, space="PSUM") as pspool,
            tc.tile_pool(name="dr", bufs=1, space="DRAM") as dpool,
        ):
            it16 = cpool.tile([P, P], mybir.dt.int16)
            nc.gpsimd.iota(it16[:], pattern=[[1, P]], channel_multiplier=0)
            it = cpool.tile([P, P], F16)
            nc.vector.tensor_copy(out=it[:], in_=it16[:])

            idx16 = cpool.tile([P, M], mybir.dt.int16)
            nc.sync.dma_start(out=idx16[:], in_=ai[:, :])
            idx32 = cpool.tile([P, M], mybir.dt.int32)
            nc.vector.tensor_copy(out=idx32[:], in_=idx16[:])
            ad8 = cpool.tile([P, M], mybir.dt.uint8)
            nc.sync.dma_start(out=ad8[:], in_=ad[:, :])
            adf = cpool.tile([P, M], F16)
            nc.vector.tensor_copy(out=adf[:], in_=ad8[:])

            partial = dpool.tile([NPAD, EW], F16)
            rsout = dpool.tile([NRC, EW], F16)

            for b0, nb, t0, tb in batches:
                gtile = gpool.tile([P, TBM * EW], F16, tag="gt")
                for k in range(tb):
                    nc.gpsimd.indirect_dma_start(
                        out=gtile[:, k * EW : (k + 1) * EW],
                        out_offset=None,
                        in_=gt[:, :],
                        in_offset=bass.IndirectOffsetOnAxis(
                            ap=idx32[:, t0 + k : t0 + k + 1], axis=0
                        ),
                    )
                sb = spool.tile([P, TBM * P], F16, tag="oh")
                nc.any.tensor_tensor(
                    out=sb[:, 0 : tb * P].rearrange("p (m q) -> p m q", q=P),
                    in0=it[:].unsqueeze(1).to_broadcast([P, tb, P]),
                    in1=adf[:, t0 : t0 + tb].unsqueeze(2).to_broadcast(
                        [P, tb, P]
                    ),
                    op=mybir.AluOpType.is_equal,
                )
                for bi in range(nb):
                    b = b0 + bi
                    kb = int(Kb[b])
                    m0 = int(taskofs[b]) - t0
                    ps = pspool.tile([P, EW], mybir.dt.float32, tag="ps")
                    for k in range(kb):
                        nc.tensor.matmul(
                            out=ps[:, :],
                            lhsT=sb[:, (m0 + k) * P : (m0 + k + 1) * P],
                            rhs=gtile[:, (m0 + k) * EW : (m0 + k + 1) * EW],
                            start=(k == 0),
                            stop=(k == kb - 1),
                        )
                    ob = opool.tile([P, EW], F16, tag="ob")
                    nc.scalar.copy(out=ob[:], in_=ps[:, :])
                    nc.sync.dma_start(
                        out=partial[b * P : (b + 1) * P, :], in_=ob[:]
                    )

            nc.gpsimd.collective_compute(
                "ReduceScatter",
                mybir.AluOpType.add,
                replica_groups=[list(range(N_CORES))],
                ins=[partial[:].opt()],
                outs=[rsout[:].opt()],
            )
            nc.sync.dma_start(out=outt[:, :], in_=rsout[:, :])
    nc.compile()
    return nc


def _run(h, h_attn_q, W_attn, b_attn, edge_index, **spmd_kwargs):
    global last_results
    gtab, aidx, adst, Kb, taskofs, M = _preprocess(h, W_attn, edge_index)
    nc = _build_program(M, Kb, taskofs)
    in_maps = [
        {"gtab": gtab[c], "aidx": aidx[c], "adst": adst[c]}
        for c in range(N_CORES)
    ]
    res = run_bass_kernel_spmd(
        nc, in_maps, core_ids=list(range(N_CORES)), **spmd_kwargs
    )
    last_results = res
    import os

    if os.environ.get("GNN_TIME2"):
        import time as _time

        global last_exec_s
        t0 = _time.time()
        res = run_bass_kernel_spmd(
            nc, in_maps, core_ids=list(range(N_CORES)), **spmd_kwargs
        )
        last_exec_s = _time.time() - t0
        last_results = res

    full = np.empty((NPAD, EW), dtype=np.float16)
    for c in range(N_CORES):
        full[c * NRC : (c + 1) * NRC] = np.asarray(res.results[c]["outt"])
    num = full[:N_NODES, :D].astype(np.float32)
    den = full[:N_NODES, D].astype(np.float32)
    return num / (den[:, None] + 1e-16)


def kernel(h, h_attn_q, W_attn, b_attn, edge_index):
    return _run(h, h_attn_q, W_attn, b_attn, edge_index)


# revision 7
# speedup vs baseline: 6.3281x; 1.4035x over previous
"""AttnConv (GNN message passing) Trainium2 kernel — src-sharded edge-parallel.

Math: out[i] = sum_{e: dst_e=i} a_e * h[src_e], a = scatter-softmax(scores, dst),
scores = alpha_q[dst] + alpha_k[src] + b.  Within one dst group alpha_q[dst]+b is
constant and cancels in the softmax, so with w = exp(alpha_k - max alpha_k):
    out[i] = (sum_e w[src_e] * h[src_e]) / (sum_e w[src_e])

Sharding (the axon host<->device tunnel is ~20-50 MB/s, so bytes moved per run
dominate):  edges are assigned to the core that OWNS their src row.  Each core
holds only its 1/8 slice of the gather table [w*h (f8 e4m3), w (f16, packed as
2 raw bytes)], gathers one 68-byte row per edge, scatter-adds into a full-size
partial [100352, 65] f16 via one-hot matmuls (PSUM f32), then one f16
ReduceScatter(add) leaves each core with its dst range [c*12544, (c+1)*12544),
which it divides (num/den) on device and returns as f16 [12544, 64].

Per-edge metadata is compressed: local src index int16 + dst offset uint8,
decompressed on device.  Edges are sorted by (dst block, src); each 128-node
dst block gets ceil(max_core_count/128) chunks of 128 edge slots; pad slots
point at a zeroed table row so they contribute nothing.
"""

import os

import numpy as np

import jax

try:
    # the SPMD runner re-jits its wrapper on every call; the persistent
    # cache turns the repeated XLA+BIR->NEFF compile into a ~0.1s lookup
    jax.config.update(
        "jax_compilation_cache_dir",
        os.path.expanduser("~/.cache/jax-bass-cache"),
    )
    jax.config.update("jax_persistent_cache_min_entry_size_bytes", -1)
    jax.config.update("jax_persistent_cache_min_compile_time_secs", 0.0)
except Exception:
    pass

import concourse.bacc as bacc
import concourse.bass as bass
import concourse.tile as tile
from concourse import mybir
from concourse.bass_utils import run_bass_kernel_spmd

N_NODES = 100000
D = 64
N_CORES = 8
P = 128
NC_SRC = N_NODES // N_CORES          # 12500 table rows owned per core
NR = 12544                           # table rows padded (98 * 128)
PADROW = NR - 1                      # zeroed row used by pad slots
NRC = 12544                          # dst nodes per core chunk (= NPAD / 8)
NPAD = NRC * N_CORES                 # 100352 padded dst space
NBLK = NPAD // P                     # 784 dst blocks
GW = 66                              # f16 table row: 64 w*h + w + pad
EW = 65                              # accumulated row: 64 w*h + w
GB = 8                               # dst blocks per work batch

F16 = mybir.dt.float16

last_results = None  # BassKernelResults of the most recent run (test harness)


def _preprocess(h, W_attn, edge_index):
    h = np.asarray(h, dtype=np.float32)
    W = np.asarray(W_attn, dtype=np.float32)
    src = np.asarray(edge_index[0]).astype(np.int64)
    dst = np.asarray(edge_index[1]).astype(np.int64)

    alpha = h @ W[D:, 0]
    w = np.exp(alpha - alpha.max(), dtype=np.float32)
    wh = h * w[:, None]
    S = 1.0
    gtab = np.zeros((N_CORES, NR, GW), dtype=np.float16)
    gtab[:, :NC_SRC, :D] = wh.astype(np.float16).reshape(N_CORES, NC_SRC, D)
    gtab[:, :NC_SRC, D] = w.astype(np.float16).reshape(N_CORES, NC_SRC)

    core = src // NC_SRC
    blk = dst >> 7
    key = core * NBLK + blk
    order = np.lexsort((src, key))
    key_s = key[order]
    srcl_s = (src[order] - core[order] * NC_SRC).astype(np.int16)
    off_s = (dst[order] & 127).astype(np.uint8)

    cnt = np.bincount(key_s, minlength=N_CORES * NBLK).reshape(N_CORES, NBLK)
    Kb = np.maximum(1, -(-cnt.max(axis=0) // P)).astype(np.int64)  # [NBLK]
    taskofs = np.zeros(NBLK + 1, dtype=np.int64)
    np.cumsum(Kb, out=taskofs[1:])
    M = int(taskofs[-1])

    cstart = np.zeros(N_CORES * NBLK, dtype=np.int64)
    np.cumsum(cnt.reshape(-1)[:-1], out=cstart[1:])
    rank = np.arange(key_s.shape[0], dtype=np.int64) - cstart[key_s]
    slot = (taskofs[key_s % NBLK] << 7) + rank
    core_s = key_s // NBLK

    aidx = np.full((N_CORES, M * P), PADROW, dtype=np.int16)
    adst = np.zeros((N_CORES, M * P), dtype=np.uint8)
    aidx[core_s, slot] = srcl_s
    adst[core_s, slot] = off_s
    aidx = np.ascontiguousarray(aidx.reshape(N_CORES, M, P).transpose(0, 2, 1))
    adst = np.ascontiguousarray(adst.reshape(N_CORES, M, P).transpose(0, 2, 1))
    return gtab, aidx, adst, Kb, taskofs, M, S


def _build_program(M, Kb, taskofs, S):
    nc = bacc.Bacc(
        "TRN2",
        target_bir_lowering=False,
        debug=False,
        enable_asserts=False,
        num_devices=N_CORES,
    )
    gt = nc.dram_tensor("gtab", [NR, GW], F16, kind="ExternalInput")
    ai = nc.dram_tensor("aidx", [P, M], mybir.dt.int16, kind="ExternalInput")
    ad = nc.dram_tensor("adst", [P, M], mybir.dt.uint8, kind="ExternalInput")
    outt = nc.dram_tensor("outt", [NRC, D], F16, kind="ExternalOutput")

    batches = []  # (b0, nb, t0, tb)
    for b0 in range(0, NBLK, GB):
        nb = min(GB, NBLK - b0)
        t0 = int(taskofs[b0])
        tb = int(taskofs[b0 + nb] - t0)
        batches.append((b0, nb, t0, tb))
    TBM = max(tb for _, _, _, tb in batches)

    with tile.TileContext(nc) as tc:
        with (
            tc.tile_pool(name="const", bufs=1) as cpool,
            tc.tile_pool(name="gath", bufs=3) as gpool,
            tc.tile_pool(name="oneh", bufs=3) as spool,
            tc.tile_pool(name="ob", bufs=4) as opool,
            tc.tile_pool(name="ps", bufs=8, space="PSUM") as pspool,
            tc.tile_pool(name="dr", bufs=1, space="DRAM") as dpool,
        ):
            it16 = cpool.tile([P, P], mybir.dt.int16)
            nc.gpsimd.iota(it16[:], pattern=[[1, P]], channel_multiplier=0)
            it = cpool.tile([P, P], F16)
            nc.vector.tensor_copy(out=it[:], in_=it16[:])

            idx16 = cpool.tile([P, M], mybir.dt.int16)
            nc.sync.dma_start(out=idx16[:], in_=ai[:, :])
            idx32 = cpool.tile([P, M], mybir.dt.int32)
            nc.vector.tensor_copy(out=idx32[:], in_=idx16[:])
            ad8 = cpool.tile([P, M], mybir.dt.uint8)
            nc.sync.dma_start(out=ad8[:], in_=ad[:, :])
            adf = cpool.tile([P, M], F16)
            nc.vector.tensor_copy(out=adf[:], in_=ad8[:])

            partial = dpool.tile([NPAD, EW], F16)
            rsout = dpool.tile([NRC, EW], F16)

            for b0, nb, t0, tb in batches:
                gtile = gpool.tile([P, TBM * GW], F16, tag="gt")
                for k in range(tb):
                    nc.gpsimd.indirect_dma_start(
                        out=gtile[:, k * GW : (k + 1) * GW],
                        out_offset=None,
                        in_=gt[:, :],
                        in_offset=bass.IndirectOffsetOnAxis(
                            ap=idx32[:, t0 + k : t0 + k + 1], axis=0
                        ),
                    )
                sb = spool.tile([P, TBM * P], F16, tag="oh")
                nc.any.tensor_tensor(
                    out=sb[:, 0 : tb * P].rearrange("p (m q) -> p m q", q=P),
                    in0=it[:].unsqueeze(1).to_broadcast([P, tb, P]),
                    in1=adf[:, t0 : t0 + tb].unsqueeze(2).to_broadcast(
                        [P, tb, P]
                    ),
                    op=mybir.AluOpType.is_equal,
                )
                for bi in range(nb):
                    b = b0 + bi
                    kb = int(Kb[b])
                    m0 = int(taskofs[b]) - t0
                    ps = pspool.tile([P, EW], mybir.dt.float32, tag="ps")
                    for k in range(kb):
                        nc.tensor.matmul(
                            out=ps[:, :],
                            lhsT=sb[:, (m0 + k) * P : (m0 + k + 1) * P],
                            rhs=gtile[:, (m0 + k) * GW : (m0 + k) * GW + EW],
                            start=(k == 0),
                            stop=(k == kb - 1),
                        )
                    ob = opool.tile([P, EW], F16, tag="ob")
                    nc.scalar.copy(out=ob[:], in_=ps[:, :])
                    nc.sync.dma_start(
                        out=partial[b * P : (b + 1) * P, :], in_=ob[:]
                    )

            nc.gpsimd.collective_compute(
                "ReduceScatter",
                mybir.AluOpType.add,
                replica_groups=[list(range(N_CORES))],
                ins=[partial[:].opt()],
                outs=[rsout[:].opt()],
            )

            # divide num/den per 128-row tile, emit f16 [NRC, 64]
            for r0 in range(0, NRC, P):
                t = opool.tile([P, EW], F16, tag="dv")
                nc.sync.dma_start(out=t[:], in_=rsout[r0 : r0 + P, :])
                den = opool.tile([P, 1], mybir.dt.float32, tag="dn")
                nc.vector.tensor_scalar(
                    out=den[:],
                    in0=t[:, D : D + 1],
                    scalar1=S,
                    scalar2=1e-12,
                    op0=mybir.AluOpType.mult,
                    op1=mybir.AluOpType.max,
                )
                rec = opool.tile([P, 1], mybir.dt.float32, tag="rc")
                nc.vector.reciprocal(out=rec[:], in_=den[:])
                of = opool.tile([P, D], F16, tag="of")
                nc.vector.tensor_tensor(
                    out=of[:],
                    in0=t[:, 0:D],
                    in1=rec[:].to_broadcast([P, D]),
                    op=mybir.AluOpType.mult,
                )
                nc.sync.dma_start(out=outt[r0 : r0 + P, :], in_=of[:])
    nc.compile()
    return nc


def _run(h, h_attn_q, W_attn, b_attn, edge_index, **spmd_kwargs):
    global last_results
    gtab, aidx, adst, Kb, taskofs, M, S = _preprocess(h, W_attn, edge_index)
    nc = _build_program(M, Kb, taskofs, S)
    in_maps = [
        {"gtab": gtab[c], "aidx": aidx[c], "adst": adst[c]}
        for c in range(N_CORES)
    ]
    res = run_bass_kernel_spmd(
        nc, in_maps, core_ids=list(range(N_CORES)), **spmd_kwargs
    )
    last_results = res
    import os

    if os.environ.get("GNN_TIME2"):
        import time as _time

        global last_exec_s
        t0 = _time.time()
        res = run_bass_kernel_spmd(
            nc, in_maps, core_ids=list(range(N_CORES)), **spmd_kwargs
        )
        last_exec_s = _time.time() - t0
        last_results = res

    full = np.empty((NPAD, D), dtype=np.float16)
    for c in range(N_CORES):
        full[c * NRC : (c + 1) * NRC] = np.asarray(res.results[c]["outt"])
    return full[:N_NODES].astype(np.float32)


def kernel(h, h_attn_q, W_attn, b_attn, edge_index):
    return _run(h, h_attn_q, W_attn, b_attn, edge_index)


# revision 8
# speedup vs baseline: 6.6915x; 1.0574x over previous
"""AttnConv (GNN message passing) Trainium2 kernel — src-sharded edge-parallel.

Math: out[i] = sum_{e: dst_e=i} a_e * h[src_e], a = scatter-softmax(scores, dst),
scores = alpha_q[dst] + alpha_k[src] + b.  Within one dst group alpha_q[dst]+b is
constant and cancels in the softmax, so with w = exp(alpha_k - max alpha_k):
    out[i] = (sum_e w[src_e] * h[src_e]) / (sum_e w[src_e])

Sharding (the axon host<->device tunnel is ~20-50 MB/s, so bytes moved per run
dominate):  edges are assigned to the core that OWNS their src row.  Each core
holds only its 1/8 slice of the gather table [w*h (f8 e4m3), w (f16, packed as
2 raw bytes)], gathers one 68-byte row per edge, scatter-adds into a full-size
partial [100352, 65] f16 via one-hot matmuls (PSUM f32), then one f16
ReduceScatter(add) leaves each core with its dst range [c*12544, (c+1)*12544),
which it divides (num/den) on device and returns as f16 [12544, 64].

Per-edge metadata is compressed: local src index int16 + dst offset uint8,
decompressed on device.  Edges are sorted by (dst block, src); each 128-node
dst block gets ceil(max_core_count/128) chunks of 128 edge slots; pad slots
point at a zeroed table row so they contribute nothing.
"""

import os

import numpy as np

import jax

try:
    # the SPMD runner re-jits its wrapper on every call; the persistent
    # cache turns the repeated XLA+BIR->NEFF compile into a ~0.1s lookup
    jax.config.update(
        "jax_compilation_cache_dir",
        os.path.expanduser("~/.cache/jax-bass-cache"),
    )
    jax.config.update("jax_persistent_cache_min_entry_size_bytes", -1)
    jax.config.update("jax_persistent_cache_min_compile_time_secs", 0.0)
except Exception:
    pass

import concourse.bacc as bacc
import concourse.bass as bass
import concourse.tile as tile
from concourse import mybir
from concourse.bass_utils import run_bass_kernel_spmd

N_NODES = 100000
D = 64
N_CORES = 8
P = 128
NC_SRC = N_NODES // N_CORES          # 12500 table rows owned per core
NR = 12544                           # table rows padded (98 * 128)
PADROW = NR - 1                      # zeroed row used by pad slots
NRC = 12544                          # dst nodes per core chunk (= NPAD / 8)
NPAD = NRC * N_CORES                 # 100352 padded dst space
NBLK = NPAD // P                     # 784 dst blocks
GW = 66                              # f16 table row: 64 w*h + w + pad
EW = 65                              # accumulated row: 64 w*h + w
GB = 8                               # dst blocks per work batch

F16 = mybir.dt.float16

last_results = None  # BassKernelResults of the most recent run (test harness)


def _preprocess(h, W_attn, edge_index):
    h = np.asarray(h, dtype=np.float32)
    W = np.asarray(W_attn, dtype=np.float32)
    src = np.asarray(edge_index[0]).astype(np.int64)
    dst = np.asarray(edge_index[1]).astype(np.int64)

    alpha = h @ W[D:, 0]
    w = np.exp(alpha - alpha.max(), dtype=np.float32)
    wh = h * w[:, None]
    S = 1.0
    gtab = np.zeros((N_CORES, NR, GW), dtype=np.float16)
    gtab[:, :NC_SRC, :D] = wh.astype(np.float16).reshape(N_CORES, NC_SRC, D)
    gtab[:, :NC_SRC, D] = w.astype(np.float16).reshape(N_CORES, NC_SRC)

    core = src // NC_SRC
    blk = dst >> 7
    key = core * NBLK + blk
    order = np.lexsort((src, key))
    key_s = key[order]
    srcl_s = (src[order] - core[order] * NC_SRC).astype(np.int16)
    off_s = (dst[order] & 127).astype(np.uint8)

    cnt = np.bincount(key_s, minlength=N_CORES * NBLK).reshape(N_CORES, NBLK)
    Kb = np.maximum(1, -(-cnt.max(axis=0) // P)).astype(np.int64)  # [NBLK]
    taskofs = np.zeros(NBLK + 1, dtype=np.int64)
    np.cumsum(Kb, out=taskofs[1:])
    M = int(taskofs[-1])

    cstart = np.zeros(N_CORES * NBLK, dtype=np.int64)
    np.cumsum(cnt.reshape(-1)[:-1], out=cstart[1:])
    rank = np.arange(key_s.shape[0], dtype=np.int64) - cstart[key_s]
    slot = (taskofs[key_s % NBLK] << 7) + rank
    core_s = key_s // NBLK

    aidx = np.full((N_CORES, M * P), PADROW, dtype=np.int16)
    adst = np.zeros((N_CORES, M * P), dtype=np.uint8)
    aidx[core_s, slot] = srcl_s
    adst[core_s, slot] = off_s
    aidx = np.ascontiguousarray(aidx.reshape(N_CORES, M, P).transpose(0, 2, 1))
    adst = np.ascontiguousarray(adst.reshape(N_CORES, M, P).transpose(0, 2, 1))
    return gtab, aidx, adst, Kb, taskofs, M, S


def _build_program(M, Kb, taskofs, S):
    nc = bacc.Bacc(
        "TRN2",
        target_bir_lowering=False,
        debug=False,
        enable_asserts=False,
        num_devices=N_CORES,
    )
    gt = nc.dram_tensor("gtab", [NR, GW], F16, kind="ExternalInput")
    ai = nc.dram_tensor("aidx", [P, M], mybir.dt.int16, kind="ExternalInput")
    ad = nc.dram_tensor("adst", [P, M], mybir.dt.uint8, kind="ExternalInput")
    outt = nc.dram_tensor("outt", [NRC, D], F16, kind="ExternalOutput")

    batches = []  # (b0, nb, t0, tb)
    for b0 in range(0, NBLK, GB):
        nb = min(GB, NBLK - b0)
        t0 = int(taskofs[b0])
        tb = int(taskofs[b0 + nb] - t0)
        batches.append((b0, nb, t0, tb))
    TBM = max(tb for _, _, _, tb in batches)

    with tile.TileContext(nc) as tc:
        with (
            tc.tile_pool(name="const", bufs=1) as cpool,
            tc.tile_pool(name="gath", bufs=3) as gpool,
            tc.tile_pool(name="oneh", bufs=3) as spool,
            tc.tile_pool(name="ob", bufs=4) as opool,
            tc.tile_pool(name="ps", bufs=8, space="PSUM") as pspool,
            tc.tile_pool(name="dr", bufs=1, space="DRAM") as dpool,
        ):
            it16 = cpool.tile([P, P], mybir.dt.int16)
            nc.gpsimd.iota(it16[:], pattern=[[1, P]], channel_multiplier=0)
            it = cpool.tile([P, P], F16)
            nc.vector.tensor_copy(out=it[:], in_=it16[:])

            idx16 = cpool.tile([P, M], mybir.dt.int16)
            nc.sync.dma_start(out=idx16[:], in_=ai[:, :])
            idx32 = cpool.tile([P, M], mybir.dt.int32)
            nc.vector.tensor_copy(out=idx32[:], in_=idx16[:])
            ad8 = cpool.tile([P, M], mybir.dt.uint8)
            nc.sync.dma_start(out=ad8[:], in_=ad[:, :])
            adf = cpool.tile([P, M], F16)
            nc.vector.tensor_copy(out=adf[:], in_=ad8[:])

            partial = dpool.tile([NPAD, EW], F16)
            rsout = dpool.tile([NRC, EW], F16)

            for b0, nb, t0, tb in batches:
                gtile = gpool.tile([P, TBM * GW], F16, tag="gt")
                for k in range(tb):
                    nc.gpsimd.indirect_dma_start(
                        out=gtile[:, k * GW : (k + 1) * GW],
                        out_offset=None,
                        in_=gt[:, :],
                        in_offset=bass.IndirectOffsetOnAxis(
                            ap=idx32[:, t0 + k : t0 + k + 1], axis=0
                        ),
                    )
                sb = spool.tile([P, TBM * P], F16, tag="oh")
                nc.any.tensor_tensor(
                    out=sb[:, 0 : tb * P].rearrange("p (m q) -> p m q", q=P),
                    in0=it[:].unsqueeze(1).to_broadcast([P, tb, P]),
                    in1=adf[:, t0 : t0 + tb].unsqueeze(2).to_broadcast(
                        [P, tb, P]
                    ),
                    op=mybir.AluOpType.is_equal,
                )
                for bi in range(nb):
                    b = b0 + bi
                    kb = int(Kb[b])
                    m0 = int(taskofs[b]) - t0
                    ps = pspool.tile([P, EW], mybir.dt.float32, tag="ps")
                    for k in range(kb):
                        nc.tensor.matmul(
                            out=ps[:, :],
                            lhsT=sb[:, (m0 + k) * P : (m0 + k + 1) * P],
                            rhs=gtile[:, (m0 + k) * GW : (m0 + k) * GW + EW],
                            start=(k == 0),
                            stop=(k == kb - 1),
                        )
                    ob = opool.tile([P, EW], F16, tag="ob")
                    nc.scalar.copy(out=ob[:], in_=ps[:, :])
                    nc.sync.dma_start(
                        out=partial[b * P : (b + 1) * P, :], in_=ob[:]
                    )

            nc.gpsimd.collective_compute(
                "ReduceScatter",
                mybir.AluOpType.add,
                replica_groups=[list(range(N_CORES))],
                ins=[partial[:].opt()],
                outs=[rsout[:].opt()],
            )

            # divide num/den per 128-row tile, emit f16 [NRC, 64]
            for r0 in range(0, NRC, P):
                t = opool.tile([P, EW], F16, tag="dv")
                nc.sync.dma_start(out=t[:], in_=rsout[r0 : r0 + P, :])
                den = opool.tile([P, 1], mybir.dt.float32, tag="dn")
                nc.vector.tensor_scalar(
                    out=den[:],
                    in0=t[:, D : D + 1],
                    scalar1=S,
                    scalar2=1e-12,
                    op0=mybir.AluOpType.mult,
                    op1=mybir.AluOpType.max,
                )
                rec = opool.tile([P, 1], mybir.dt.float32, tag="rc")
                nc.vector.reciprocal(out=rec[:], in_=den[:])
                of = opool.tile([P, D], F16, tag="of")
                nc.vector.tensor_tensor(
                    out=of[:],
                    in0=t[:, 0:D],
                    in1=rec[:].to_broadcast([P, D]),
                    op=mybir.AluOpType.mult,
                )
                nc.sync.dma_start(out=outt[r0 : r0 + P, :], in_=of[:])
    nc.compile()
    return nc


def _run(h, h_attn_q, W_attn, b_attn, edge_index, **spmd_kwargs):
    global last_results
    import time as _time

    _t0 = _time.time()
    gtab, aidx, adst, Kb, taskofs, M, S = _preprocess(h, W_attn, edge_index)
    _t1 = _time.time()
    nc = _build_program(M, Kb, taskofs, S)
    _t2 = _time.time()
    in_maps = [
        {"gtab": gtab[c], "aidx": aidx[c], "adst": adst[c]}
        for c in range(N_CORES)
    ]
    res = run_bass_kernel_spmd(
        nc, in_maps, core_ids=list(range(N_CORES)), **spmd_kwargs
    )
    last_results = res
    _t3 = _time.time()
    if os.environ.get("GNN_PHASES"):
        print(
            f"[phases] preprocess {_t1 - _t0:.2f}s build+compile "
            f"{_t2 - _t1:.2f}s run1 {_t3 - _t2:.2f}s",
            flush=True,
        )

    if os.environ.get("GNN_TIME2"):
        import time as _time

        global last_exec_s
        t0 = _time.time()
        res = run_bass_kernel_spmd(
            nc, in_maps, core_ids=list(range(N_CORES)), **spmd_kwargs
        )
        last_exec_s = _time.time() - t0
        last_results = res

    full = np.empty((NPAD, D), dtype=np.float16)
    for c in range(N_CORES):
        full[c * NRC : (c + 1) * NRC] = np.asarray(res.results[c]["outt"])
    return full[:N_NODES].astype(np.float32)


def kernel(h, h_attn_q, W_attn, b_attn, edge_index):
    return _run(h, h_attn_q, W_attn, b_attn, edge_index)


# revision 9
# speedup vs baseline: 7.3529x; 1.0988x over previous
"""AttnConv Trainium2 kernel — src-sharded edge-parallel, 512-node dst blocks.

Same math/sharding as kernel.py, but dst space is tiled in 512-node blocks:
PSUM accumulates [65 features, 512 nodes] per block (feature-major), which
quantizes edge-chunk padding per block at 128 edges against a ~1024-edge mean
(12% pad vs 50% with 128-node blocks).  Per-edge metadata: uint16 packs the
14-bit local src index with the top 2 bits of the 9-bit dst offset; uint8
carries the low 7 offset bits.  The f16 ReduceScatter runs over the
feature-major partial [8, 65, 12800]; each core divides num/den (matmul
outer-product broadcast of the f32 reciprocal) and emits f16 [64, 12800].
"""

import os

import numpy as np

import jax

try:
    jax.config.update(
        "jax_compilation_cache_dir",
        os.path.expanduser("~/.cache/jax-bass-cache"),
    )
    jax.config.update("jax_persistent_cache_min_entry_size_bytes", -1)
    jax.config.update("jax_persistent_cache_min_compile_time_secs", 0.0)
except Exception:
    pass

import concourse.bacc as bacc
import concourse.bass as bass
import concourse.tile as tile
from concourse import mybir
from concourse.bass_utils import run_bass_kernel_spmd

N_NODES = 100000
D = 64
N_CORES = 8
P = 128
NC_SRC = N_NODES // N_CORES          # 12500 table rows owned per core
NR = 12544                           # table rows padded (98 * 128)
PADROW = NR - 1                      # zeroed row used by pad slots
B5 = 512                             # dst nodes per block
NRC = 12800                          # dst nodes per core chunk (25 blocks)
NPAD = NRC * N_CORES                 # 102400 padded dst space
NBLK = NPAD // B5                    # 200 dst blocks
GW = 66                              # f16 table row: 64 w*h + w + pad
EW = 65                              # accumulated row: 64 w*h + w
GB = 2                               # dst blocks per work batch

F16 = mybir.dt.float16

last_results = None  # BassKernelResults of the most recent run (test harness)


def _preprocess(h, W_attn, edge_index):
    h = np.asarray(h, dtype=np.float32)
    W = np.asarray(W_attn, dtype=np.float32)
    src = np.asarray(edge_index[0]).astype(np.int64)
    dst = np.asarray(edge_index[1]).astype(np.int64)

    alpha = h @ W[D:, 0]
    w = np.exp(alpha - alpha.max(), dtype=np.float32)
    wh = h * w[:, None]
    gtab = np.zeros((N_CORES, NR, GW), dtype=np.float16)
    gtab[:, :NC_SRC, :D] = wh.astype(np.float16).reshape(N_CORES, NC_SRC, D)
    gtab[:, :NC_SRC, D] = w.astype(np.float16).reshape(N_CORES, NC_SRC)

    core = src // NC_SRC
    blk = dst >> 9
    off = dst & (B5 - 1)
    key = core * NBLK + blk
    order = np.lexsort((src, key))
    key_s = key[order]
    srcl_s = src[order] - core[order] * NC_SRC
    off_s = off[order]

    cnt = np.bincount(key_s, minlength=N_CORES * NBLK).reshape(N_CORES, NBLK)
    Kb = np.maximum(1, -(-cnt.max(axis=0) // P)).astype(np.int64)  # [NBLK]
    taskofs = np.zeros(NBLK + 1, dtype=np.int64)
    np.cumsum(Kb, out=taskofs[1:])
    M = int(taskofs[-1])

    cstart = np.zeros(N_CORES * NBLK, dtype=np.int64)
    np.cumsum(cnt.reshape(-1)[:-1], out=cstart[1:])
    rank = np.arange(key_s.shape[0], dtype=np.int64) - cstart[key_s]
    slot = (taskofs[key_s % NBLK] << 7) + rank
    core_s = key_s // NBLK

    aidx = np.full((N_CORES, M * P), PADROW, dtype=np.uint16)
    adst = np.zeros((N_CORES, M * P), dtype=np.uint8)
    aidx[core_s, slot] = (srcl_s | ((off_s >> 7) << 14)).astype(np.uint16)
    adst[core_s, slot] = (off_s & 127).astype(np.uint8)
    aidx = np.ascontiguousarray(aidx.reshape(N_CORES, M, P).transpose(0, 2, 1))
    adst = np.ascontiguousarray(adst.reshape(N_CORES, M, P).transpose(0, 2, 1))
    return gtab, aidx, adst, Kb, taskofs, M


def _build_program(M, Kb, taskofs):
    nc = bacc.Bacc(
        "TRN2",
        target_bir_lowering=False,
        debug=False,
        enable_asserts=False,
        num_devices=N_CORES,
    )
    gt = nc.dram_tensor("gtab", [NR, GW], F16, kind="ExternalInput")
    ai = nc.dram_tensor("aidx", [P, M], mybir.dt.uint16, kind="ExternalInput")
    ad = nc.dram_tensor("adst", [P, M], mybir.dt.uint8, kind="ExternalInput")
    outt = nc.dram_tensor("outt", [D, NRC], F16, kind="ExternalOutput")

    batches = []  # (b0, nb, t0, tb)
    for b0 in range(0, NBLK, GB):
        nb = min(GB, NBLK - b0)
        t0 = int(taskofs[b0])
        tb = int(taskofs[b0 + nb] - t0)
        batches.append((b0, nb, t0, tb))
    TBM = max(tb for _, _, _, tb in batches)

    with tile.TileContext(nc) as tc:
        with (
            tc.tile_pool(name="const", bufs=1) as cpool,
            tc.tile_pool(name="gath", bufs=3) as gpool,
            tc.tile_pool(name="oneh", bufs=3) as spool,
            tc.tile_pool(name="ob", bufs=4) as opool,
            tc.tile_pool(name="ps", bufs=6, space="PSUM") as pspool,
            tc.tile_pool(name="psb", bufs=2, space="PSUM") as pbpool,
            tc.tile_pool(name="dr", bufs=1, space="DRAM") as dpool,
        ):
            it16 = cpool.tile([P, B5], mybir.dt.int16)
            nc.gpsimd.iota(it16[:], pattern=[[1, B5]], channel_multiplier=0)
            it = cpool.tile([P, B5], F16)
            nc.vector.tensor_copy(out=it[:], in_=it16[:])
            ones = cpool.tile([1, D], F16)
            nc.vector.memset(ones[:], 1.0)

            u16 = cpool.tile([P, M], mybir.dt.uint16)
            nc.sync.dma_start(out=u16[:], in_=ai[:, :])
            u32 = cpool.tile([P, M], mybir.dt.int32)
            nc.vector.tensor_copy(out=u32[:], in_=u16[:])
            idx32 = cpool.tile([P, M], mybir.dt.int32)
            nc.vector.tensor_scalar(
                out=idx32[:],
                in0=u32[:],
                scalar1=16383,
                scalar2=None,
                op0=mybir.AluOpType.bitwise_and,
            )
            ad8 = cpool.tile([P, M], mybir.dt.uint8)
            nc.sync.dma_start(out=ad8[:], in_=ad[:, :])
            o32 = cpool.tile([P, M], mybir.dt.int32)
            nc.vector.tensor_copy(out=o32[:], in_=ad8[:])
            # off512 = (u >> 14) * 128 + low7  ==  ((u >> 14) << 7) | low7
            sh = cpool.tile([P, M], mybir.dt.int32)
            nc.vector.tensor_scalar(
                out=sh[:],
                in0=u32[:],
                scalar1=14,
                scalar2=7,
                op0=mybir.AluOpType.logical_shift_right,
                op1=mybir.AluOpType.logical_shift_left,
            )
            nc.vector.tensor_tensor(
                out=o32[:], in0=o32[:], in1=sh[:], op=mybir.AluOpType.add
            )
            adf = cpool.tile([P, M], F16)
            nc.vector.tensor_copy(out=adf[:], in_=o32[:])

            partial = dpool.tile([N_CORES, EW, NRC], F16)
            rsout = dpool.tile([EW, NRC], F16)

            for b0, nb, t0, tb in batches:
                gtile = gpool.tile([P, TBM * GW], F16, tag="gt")
                for k in range(tb):
                    nc.gpsimd.indirect_dma_start(
                        out=gtile[:, k * GW : (k + 1) * GW],
                        out_offset=None,
                        in_=gt[:, :],
                        in_offset=bass.IndirectOffsetOnAxis(
                            ap=idx32[:, t0 + k : t0 + k + 1], axis=0
                        ),
                    )
                sb = spool.tile([P, TBM * B5], F16, tag="oh")
                nc.any.tensor_tensor(
                    out=sb[:, 0 : tb * B5].rearrange("p (m q) -> p m q", q=B5),
                    in0=it[:].unsqueeze(1).to_broadcast([P, tb, B5]),
                    in1=adf[:, t0 : t0 + tb].unsqueeze(2).to_broadcast(
                        [P, tb, B5]
                    ),
                    op=mybir.AluOpType.is_equal,
                )
                for bi in range(nb):
                    b = b0 + bi
                    kb = int(Kb[b])
                    m0 = int(taskofs[b]) - t0
                    ps = pspool.tile([EW, B5], mybir.dt.float32, tag="ps")
                    for k in range(kb):
                        nc.tensor.matmul(
                            out=ps[:, :],
                            lhsT=gtile[:, (m0 + k) * GW : (m0 + k) * GW + EW],
                            rhs=sb[:, (m0 + k) * B5 : (m0 + k + 1) * B5],
                            start=(k == 0),
                            stop=(k == kb - 1),
                        )
                    ob = opool.tile([EW, B5], F16, tag="ob")
                    nc.scalar.copy(out=ob[:], in_=ps[:, :])
                    c5 = b // (NRC // B5)
                    col = (b % (NRC // B5)) * B5
                    nc.sync.dma_start(
                        out=partial[c5, :, col : col + B5], in_=ob[:]
                    )

            nc.gpsimd.collective_compute(
                "ReduceScatter",
                mybir.AluOpType.add,
                replica_groups=[list(range(N_CORES))],
                ins=[partial[:].opt()],
                outs=[rsout[:].opt()],
            )

            # out[:, j] = num[:, j] * (1/den[j]), broadcast across partitions
            # via a contract-1 matmul outer product
            for col in range(0, NRC, B5):
                t = opool.tile([EW, B5], F16, tag="dv")
                nc.sync.dma_start(out=t[:], in_=rsout[:, col : col + B5])
                den = opool.tile([1, B5], mybir.dt.float32, tag="dn")
                nc.vector.tensor_scalar(
                    out=den[:],
                    in0=t[D : D + 1, :],
                    scalar1=1e-12,
                    scalar2=None,
                    op0=mybir.AluOpType.max,
                )
                rec = opool.tile([1, B5], mybir.dt.float32, tag="rc")
                nc.vector.reciprocal(out=rec[:], in_=den[:])
                rec16 = opool.tile([1, B5], F16, tag="r6")
                nc.vector.tensor_copy(out=rec16[:], in_=rec[:])
                rb = pbpool.tile([D, B5], mybir.dt.float32, tag="rb")
                nc.tensor.matmul(
                    out=rb[:, :],
                    lhsT=ones[:],
                    rhs=rec16[:],
                    start=True,
                    stop=True,
                )
                of = opool.tile([D, B5], F16, tag="of")
                nc.vector.tensor_tensor(
                    out=of[:],
                    in0=t[0:D, :],
                    in1=rb[:, :],
                    op=mybir.AluOpType.mult,
                )
                nc.sync.dma_start(out=outt[:, col : col + B5], in_=of[:])
    nc.compile()
    return nc


def _run(h, h_attn_q, W_attn, b_attn, edge_index, **spmd_kwargs):
    global last_results
    import time as _time

    _t0 = _time.time()
    gtab, aidx, adst, Kb, taskofs, M = _preprocess(h, W_attn, edge_index)
    _t1 = _time.time()
    nc = _build_program(M, Kb, taskofs)
    _t2 = _time.time()
    in_maps = [
        {"gtab": gtab[c], "aidx": aidx[c], "adst": adst[c]}
        for c in range(N_CORES)
    ]
    res = run_bass_kernel_spmd(
        nc, in_maps, core_ids=list(range(N_CORES)), **spmd_kwargs
    )
    last_results = res
    _t3 = _time.time()
    if os.environ.get("GNN_PHASES"):
        print(
            f"[phases] preprocess {_t1 - _t0:.2f}s build+compile "
            f"{_t2 - _t1:.2f}s run1 {_t3 - _t2:.2f}s",
            flush=True,
        )

    if os.environ.get("GNN_TIME2"):
        global last_exec_s
        t0 = _time.time()
        res = run_bass_kernel_spmd(
            nc, in_maps, core_ids=list(range(N_CORES)), **spmd_kwargs
        )
        last_exec_s = _time.time() - t0
        last_results = res

    full = np.empty((D, NPAD), dtype=np.float16)
    for c in range(N_CORES):
        full[:, c * NRC : (c + 1) * NRC] = np.asarray(res.results[c]["outt"])
    return np.ascontiguousarray(full[:, :N_NODES].T).astype(np.float32)


def kernel(h, h_attn_q, W_attn, b_attn, edge_index):
    return _run(h, h_attn_q, W_attn, b_attn, edge_index)
